# revision 2
# baseline (speedup 1.0000x reference)
"""Causal multi-head attention (B=8, T=1024, E=768, H=12, D=64) on 8 trn2
NeuronCores, data-parallel over the batch (one batch element per core).

Per-core pipeline (all matmuls in float32r — full PE rate, ~1e-4 rel err):
  1. Q^T = Wq @ x^T + bq, K^T likewise  -> SBUF [768, 1024] (e_out on partitions)
  2. V = x @ Wv^T (bias folded into the output projection) -> SBUF [1024, 12*65]
     with a ones column appended per head ("V65") so the attention-context
     matmul also produces the softmax denominator.
  3. Per head pair (2 heads share a 128-partition chunk):
     S^T[k,q] tiles via row-packed matmuls (K=64 contraction, tile_position
     (0,0)/(64,0)), causal mask added on the 128-wide diagonal block only,
     exp on ACT (scale=1/8), fully-masked columns zeroed, then
     ctx^T[65,q] accumulated over k-tiles with V65 as the stationary operand.
     Row 64 of ctx^T is the softmax denominator; reciprocal is broadcast
     across partitions via a DRAM bounce and multiplied in.
  4. out = ctx_norm @ Wo^T + bo_eff where bo_eff = bo + bv @ Wo^T.

The sharding/gather and all host-side layout prep (transposes, bias
precomputation) happen in kernel() below.
"""
import sys
import numpy as np

sys.path.insert(0, "/opt/trn_rl_repo")

import concourse.bass as bass
import concourse.mybir as mybir
import concourse.tile as tile

F32 = mybir.dt.float32
F32R = mybir.dt.float32r

B, T, E, H, D = 8, 1024, 768, 12, 64
NCH = E // 128          # 6 e-chunks
NTC = T // 128          # 8 t-chunks
NW = T // 512           # 2 q-windows
SCALE = 1.0 / np.sqrt(D)
NEG = -1.0e9


def _split_excess_waits(nc, max_waits: int = 1):
    """walrus on this stack accepts at most one embedded sync-wait per
    instruction; peel extras onto wait-only NoOps on the same engine."""
    for func in nc.m.functions:
        for bb in func.blocks:
            insts = bb.instructions
            i = 0
            while i < len(insts):
                inst = insts[i]
                si = getattr(inst, "sync_info", None)
                if si is None or len(si.on_wait) <= max_waits:
                    i += 1
                    continue
                waits = list(si.on_wait)
                keep, extra = waits[:max_waits], waits[max_waits:]
                nops = []
                while extra:
                    chunk, extra = extra[:max_waits], extra[max_waits:]
                    nop = mybir.InstNoOp(
                        name=f"{inst.name}_ws{len(nops)}", ins=[], outs=[])
                    nop.engine = inst.engine
                    nop.sync_info = mybir.SyncInfo(on_wait=chunk, on_update=[])
                    nc.register_instruction(nop, overwrite=True)
                    nops.append(nop)
                si.on_wait = keep
                for j, nop in enumerate(nops):
                    insts.insert(i + j, nop)
                i += len(nops) + 1


def build_nc():
    nc = bass.Bass()
    xT = nc.dram_tensor("xT", [E, T], F32, kind="ExternalInput")
    wqT = nc.dram_tensor("wqT", [E, E], F32, kind="ExternalInput")
    wkT = nc.dram_tensor("wkT", [E, E], F32, kind="ExternalInput")
    wvT = nc.dram_tensor("wvT", [E, E], F32, kind="ExternalInput")
    woT = nc.dram_tensor("woT", [E, E], F32, kind="ExternalInput")
    bq_pm = nc.dram_tensor("bq_pm", [128, NCH], F32, kind="ExternalInput")
    bk_pm = nc.dram_tensor("bk_pm", [128, NCH], F32, kind="ExternalInput")
    bo_bc = nc.dram_tensor("bo_bc", [128, E], F32, kind="ExternalInput")
    out = nc.dram_tensor("out", [T, E], F32, kind="ExternalOutput")

    tril = np.where(np.arange(128)[None, :] >= np.arange(128)[:, None],
                    0.0, NEG).astype(np.float32)
    maskc = nc.inline_tensor(tril, name="maskc")
    ones12 = nc.inline_tensor(np.ones((128, H), np.float32), name="ones12")

    with tile.TileContext(nc) as tc:
        from contextlib import ExitStack
        with ExitStack() as ctx:
            consts = ctx.enter_context(tc.tile_pool(name="consts", bufs=1))
            persist = ctx.enter_context(tc.tile_pool(name="persist", bufs=1))
            wqk_p = ctx.enter_context(tc.tile_pool(name="wqk", bufs=3))
            wrow_p = ctx.enter_context(tc.tile_pool(name="wrow", bufs=6))
            pt_p = ctx.enter_context(tc.tile_pool(name="pt", bufs=3))
            bc_p = ctx.enter_context(tc.tile_pool(name="bc", bufs=2))
            rt_p = ctx.enter_context(tc.tile_pool(name="rt", bufs=2))
            out_p = ctx.enter_context(tc.tile_pool(name="outp", bufs=2))
            pp = ctx.enter_context(tc.tile_pool(name="pp", bufs=2, space="PSUM"))
            stp = ctx.enter_context(tc.tile_pool(name="stp", bufs=2, space="PSUM"))
            ctxp = ctx.enter_context(tc.tile_pool(name="ctxp", bufs=2, space="PSUM"))
            drp = ctx.enter_context(tc.tile_pool(name="drp", bufs=2, space="DRAM"))

            # --- constants / inputs ---
            mask_sb = consts.tile([128, 128], F32)
            nc.gpsimd.dma_start(out=mask_sb, in_=maskc[:, :])
            bqs = consts.tile([128, NCH], F32)
            nc.gpsimd.dma_start(out=bqs, in_=bq_pm[:, :])
            bks = consts.tile([128, NCH], F32)
            nc.gpsimd.dma_start(out=bks, in_=bk_pm[:, :])
            bos = consts.tile([128, E], F32)
            nc.gpsimd.dma_start(out=bos, in_=bo_bc[:, :])

            xt_sb = persist.tile([128, NCH, T], F32R)
            for ch in range(NCH):
                nc.gpsimd.dma_start(
                    out=xt_sb[:, ch, :],
                    in_=xT[ch * 128:(ch + 1) * 128, :].bitcast(F32R))

            qt_sb = persist.tile([128, NCH, T], F32R)
            kt_sb = persist.tile([128, NCH, T], F32R)
            v65_sb = persist.tile([128, NTC, H * 65], F32R)
            ctxT_sb = persist.tile([128, NCH, T], F32R)

            # v65 ones columns (one DMA per t-chunk)
            for kc in range(NTC):
                v65_r = v65_sb[:, kc, :].rearrange("p (h e) -> p h e", e=65)
                nc.gpsimd.dma_start(out=v65_r[:, :, 64:65],
                                    in_=ones12[:, :].bitcast(F32R))

            # wv tiles (row-chunks of wvT), streamed
            wv_t = {}
            for ch in range(NCH):
                w = wrow_p.tile([128, E], F32R, tag="wrow")
                nc.gpsimd.dma_start(
                    out=w, in_=wvT[ch * 128:(ch + 1) * 128, :].bitcast(F32R))
                wv_t[ch] = w

            def proj_qk(m, wT, bias_sb, dst_sb):
                w = wqk_p.tile([128, NCH, 128], F32R, tag="wqk")
                nc.gpsimd.dma_start(
                    out=w,
                    in_=wT[:, m * 128:(m + 1) * 128]
                    .rearrange("(c p) m -> p c m", p=128).bitcast(F32R))
                for win in range(NW):
                    ps = pp.tile([128, 512], F32, tag="pp")
                    for ch in range(NCH):
                        nc.tensor.matmul(
                            ps, w[:, ch, :], xt_sb[:, ch, win * 512:(win + 1) * 512],
                            start=(ch == 0), stop=(ch == NCH - 1))
                    nc.vector.tensor_scalar_add(
                        dst_sb[:, m, win * 512:(win + 1) * 512], ps,
                        bias_sb[:, m:m + 1])

            def proj_v(kc):
                ps0 = pp.tile([128, 512], F32, tag="pp")
                ps1 = pp.tile([128, 256], F32, tag="pp")
                for ch in range(NCH):
                    lhsT = xt_sb[:, ch, kc * 128:(kc + 1) * 128]
                    nc.tensor.matmul(ps0, lhsT, wv_t[ch][:, 0:512],
                                     start=(ch == 0), stop=(ch == NCH - 1))
                    nc.tensor.matmul(ps1, lhsT, wv_t[ch][:, 512:768],
                                     start=(ch == 0), stop=(ch == NCH - 1))
                v65_r = v65_sb[:, kc, :].rearrange("p (h e) -> p h e", e=65)
                nc.vector.tensor_copy(v65_r[:, 0:8, 0:64], ps0)
                nc.vector.tensor_copy(v65_r[:, 8:12, 0:64], ps1)

            def attn_pair(p):
                for win in range(NW):
                    nk = 4 * (win + 1)
                    ctxA = ctxp.tile([65, 512], F32, tag="ctx")
                    ctxB = ctxp.tile([65, 512], F32, tag="ctx")
                    for kc in range(nk):
                        st = stp.tile([128, 1024], F32, tag="st")
                        nc.tensor.matmul(
                            st[:, 0:512],
                            kt_sb[0:64, p, kc * 128:(kc + 1) * 128],
                            qt_sb[0:64, p, win * 512:(win + 1) * 512],
                            start=True, stop=True, tile_position=(0, 0))
                        nc.tensor.matmul(
                            st[:, 512:1024],
                            kt_sb[64:128, p, kc * 128:(kc + 1) * 128],
                            qt_sb[64:128, p, win * 512:(win + 1) * 512],
                            start=True, stop=True, tile_position=(64, 0))
                        off = kc * 128 - win * 512
                        if off >= 0:
                            nc.vector.tensor_tensor(
                                out=st[:, off:off + 128], in0=st[:, off:off + 128],
                                in1=mask_sb, op=mybir.AluOpType.add)
                            nc.vector.tensor_tensor(
                                out=st[:, 512 + off:512 + off + 128],
                                in0=st[:, 512 + off:512 + off + 128],
                                in1=mask_sb, op=mybir.AluOpType.add)
                        pt = pt_p.tile([128, 1024], F32R, tag="pt")
                        nc.scalar.activation(
                            pt, st, mybir.ActivationFunctionType.Exp, scale=SCALE)
                        if off > 0:
                            nc.vector.tensor_scalar_mul(
                                pt[:, 0:off], pt[:, 0:off], 0.0)
                            nc.vector.tensor_scalar_mul(
                                pt[:, 512:512 + off], pt[:, 512:512 + off], 0.0)
                        hA, hB = 2 * p, 2 * p + 1
                        nc.tensor.matmul(
                            ctxA, v65_sb[:, kc, hA * 65:hA * 65 + 65], pt[:, 0:512],
                            start=(kc == 0), stop=(kc == nk - 1))
                        nc.tensor.matmul(
                            ctxB, v65_sb[:, kc, hB * 65:hB * 65 + 65], pt[:, 512:1024],
                            start=(kc == 0), stop=(kc == nk - 1))
                    for idx, cps in ((0, ctxA), (1, ctxB)):
                        rt = rt_p.tile([1, 512], F32, tag="rt")
                        nc.vector.reciprocal(rt, cps[64:65, :])
                        sc = drp.tile([1, 512], F32, tag="dr")
                        nc.gpsimd.dma_start(out=sc, in_=rt)
                        bc = bc_p.tile([64, 512], F32, tag="bc")
                        sc_b = bass.AP(tensor=sc.tensor, offset=sc.offset,
                                       ap=[[0, 64]] + list(sc.ap)[1:])
                        nc.gpsimd.dma_start(out=bc, in_=sc_b)
                        nc.vector.tensor_tensor(
                            out=ctxT_sb[idx * 64:idx * 64 + 64, p,
                                        win * 512:(win + 1) * 512],
                            in0=cps[0:64, :], in1=bc, op=mybir.AluOpType.mult)

            # --- emission order: interleave projections and attention ---
            proj_qk(0, wqT, bqs, qt_sb)
            proj_qk(0, wkT, bks, kt_sb)
            for kc in range(NTC):
                proj_v(kc)
            attn_pair(0)
            for m in range(1, NCH):
                proj_qk(m, wqT, bqs, qt_sb)
                proj_qk(m, wkT, bks, kt_sb)
                attn_pair(m)

            # --- output projection ---
            wo_t = {}
            for ch in range(NCH):
                w = wrow_p.tile([128, E], F32R, tag="wrow")
                nc.gpsimd.dma_start(
                    out=w, in_=woT[ch * 128:(ch + 1) * 128, :].bitcast(F32R))
                wo_t[ch] = w
            for tcn in range(NTC):
                ps0 = pp.tile([128, 512], F32, tag="pp")
                ps1 = pp.tile([128, 256], F32, tag="pp")
                for ch in range(NCH):
                    lhsT = ctxT_sb[:, ch, tcn * 128:(tcn + 1) * 128]
                    nc.tensor.matmul(ps0, lhsT, wo_t[ch][:, 0:512],
                                     start=(ch == 0), stop=(ch == NCH - 1))
                    nc.tensor.matmul(ps1, lhsT, wo_t[ch][:, 512:768],
                                     start=(ch == 0), stop=(ch == NCH - 1))
                ot = out_p.tile([128, E], F32, tag="outp")
                nc.vector.tensor_tensor(out=ot[:, 0:512], in0=ps0,
                                        in1=bos[:, 0:512], op=mybir.AluOpType.add)
                nc.vector.tensor_tensor(out=ot[:, 512:768], in0=ps1,
                                        in1=bos[:, 512:768], op=mybir.AluOpType.add)
                nc.gpsimd.dma_start(out=out[tcn * 128:(tcn + 1) * 128, :], in_=ot)

    _split_excess_waits(nc)
    return nc


_NC_CACHE = None


def _make_in_maps(x, Wq, bq, Wk, bk, Wv, bv, Wo, bo):
    wqT = np.ascontiguousarray(Wq.T.astype(np.float32))
    wkT = np.ascontiguousarray(Wk.T.astype(np.float32))
    wvT = np.ascontiguousarray(Wv.T.astype(np.float32))
    woT = np.ascontiguousarray(Wo.T.astype(np.float32))
    bq_pm = np.ascontiguousarray(bq.reshape(NCH, 128).T.astype(np.float32))
    bk_pm = np.ascontiguousarray(bk.reshape(NCH, 128).T.astype(np.float32))
    bo_eff = (bo.astype(np.float64)
              + bv.astype(np.float64) @ Wo.T.astype(np.float64)).astype(np.float32)
    bo_bc = np.ascontiguousarray(np.tile(bo_eff[None, :], (128, 1)))
    maps = []
    for b in range(B):
        xTb = np.ascontiguousarray(x[b].T.astype(np.float32))
        maps.append({"xT": xTb, "wqT": wqT, "wkT": wkT, "wvT": wvT, "woT": woT,
                     "bq_pm": bq_pm, "bk_pm": bk_pm, "bo_bc": bo_bc})
    return maps


def kernel(x, Wq, bq, Wk, bk, Wv, bv, Wo, bo):
    global _NC_CACHE
    from concourse.bass_utils import run_bass_kernel_spmd
    if _NC_CACHE is None:
        _NC_CACHE = build_nc()
    in_maps = _make_in_maps(x, Wq, bq, Wk, bk, Wv, bv, Wo, bo)
    res = run_bass_kernel_spmd(_NC_CACHE, in_maps, core_ids=list(range(B)))
    return np.stack([res.results[i]["out"] for i in range(B)], axis=0)
